# revision 3
# baseline (speedup 1.0000x reference)
"""Trainium2 Bass kernel for nn_BezierRenderer.

Renders B=16 gaussian-window "bezier" strokes onto 512x512 canvases:
  out[b] = max over 10 segments of clip((thick_b - dist(pixel, seg)) / thick_b, 0, 1)

Strategy
--------
The active area (pixels within `thick` of any segment) is ~1.6% of the dense
B*P*H*W domain, so the kernel rasterizes only per-segment bounding-box
windows.  For each segment two planes that are affine in pixel coordinates
are produced by one small fp32 PE matmul (K=2, stationary = [1; coord]):

   z  = projection coordinate / sqrt(d2+1e-5), shifted by -m/2
   w  = exact perpendicular offset  (w^2 = |p-v|^2 - s^2/d2)

With m = sqrt(d2+1e-5), the reference's clamped-projection distance is
   dist^2 = w^2 + relu(|z| - m/2)^2        (to within ~1e-10 absolute)
The per-slot pipeline is:  PE matmul -> ACT Abs + ACT Square (PSUM->SBUF)
-> DVE tensor_scalar relu (2x mode) -> DVE square -> DVE add -> DVE min
into a per-stroke-tile running-min plane.  Epilogue per tile:
ACT sqrt(minacc * invthick^2), ACT relu(1 - dist/thick), DMA out.

Work is split across the 8 NeuronCores stroke-wise (greedy balance by
estimated DVE cycles).  Each core gets its own specialized Bass program
(the sparse windows are baked in at trace time); programs run concurrently,
one per core, via PJRT with jax.default_device pinning.  The host only
mirrors the reference's tiny stroke-endpoint setup (160 control points),
packs the per-slot affine coefficients, and scatters the computed tiles
into the zero canvas.
"""

import threading
from contextlib import ExitStack

import numpy as np

# ---------------------------------------------------------------------------
# problem constants (hardcoded per contract; kernel.py must be self-contained)
# ---------------------------------------------------------------------------
SIZE = 512
NUM_CTRL = 4
P = 10  # samples per curve -> 10 segments
B = 16
N_CORES = 8
MARGIN_PAD = 1.5  # extra pixels beyond thick for bbox safety
MAX_W = 256  # max window width per matmul (PSUM bank = 512 fp32 = 2w)
INIT_MIN = 1.0e12


# ---------------------------------------------------------------------------
# host-side geometry (mirrors reference.py numerics)
# ---------------------------------------------------------------------------
def _bezier_weights():
    M = 2 * P
    n = np.arange(M) - (M - 1) / 2.0
    gaus = np.exp(-0.5 * (n / 2.0) ** 2) * 0.75
    W = np.zeros((NUM_CTRL, P), dtype=np.float32)
    for i in range(NUM_CTRL):
        start = int(P - P * (i / (NUM_CTRL - 1)))
        W[i, :] = gaus[start : start + P]
    return W


def _host_strokes(trajectories, thicknesses):
    W = _bezier_weights()
    traj = np.asarray(trajectories, dtype=np.float32)
    sample = np.einsum("bck,kp->bpc", traj, W).astype(np.float32)
    last = traj[:, :, 3][:, None, :]
    stroke = np.concatenate([sample, last], axis=1).astype(np.float32)
    stroke = stroke * np.float32(SIZE)  # (B, P+1, 2) [y, x]
    vs = stroke[:, :-1]  # (B, P, 2)
    ws = stroke[:, 1:]
    th = np.asarray(thicknesses, dtype=np.float32)[:, 0] * np.float32(2.0) + np.float32(0.5)
    thick = np.float32(2.0) * th.sum(-1, dtype=np.float32)  # (B,)
    return vs, ws, thick


# ---------------------------------------------------------------------------
# work planning: per stroke -> orientation, part-blocks, per-(seg,block) slots
# ---------------------------------------------------------------------------
class Slot:
    __slots__ = ("tile_idx", "f_lo", "f_w", "row0_z", "row1_z", "row0_w", "row1_w", "h")

    def __init__(self, tile_idx, f_lo, f_w, row0_z, row1_z, row0_w, row1_w, h):
        self.tile_idx = tile_idx
        self.f_lo = f_lo
        self.f_w = f_w
        self.row0_z = row0_z  # (f_w,) float32 - const row of z-plane (centered clamp)
        self.row1_z = row1_z  # scalar - coefficient on p_hat
        self.row0_w = row0_w
        self.row1_w = row1_w
        self.h = h  # half-width of clamp (m/2)


class Tile:
    __slots__ = ("stroke", "transposed", "p_lo", "p_ext", "f_lo", "f_ext", "invth2", "thick")

    def __init__(self, stroke, transposed, p_lo, p_ext, f_lo, f_ext, invth2, thick):
        self.stroke = stroke
        self.transposed = transposed  # True: partition axis = x, free axis = y
        self.p_lo = p_lo
        self.p_ext = p_ext
        self.f_lo = f_lo
        self.f_ext = f_ext
        self.invth2 = invth2
        self.thick = thick


def _plan_stroke(b, vs, ws, thick):
    """Returns (tiles, slots) for one stroke, or ([], []) if fully off-canvas."""
    v = vs[b].astype(np.float64)  # (P, 2) [y, x]
    w = ws[b].astype(np.float64)
    margin = float(thick[b]) + MARGIN_PAD

    lo = np.minimum(v, w).min(axis=0) - margin  # (2,)
    hi = np.maximum(v, w).max(axis=0) + margin
    ylo = max(0, int(np.floor(lo[0])))
    yhi = min(SIZE, int(np.ceil(hi[0])) + 1)
    xlo = max(0, int(np.floor(lo[1])))
    xhi = min(SIZE, int(np.ceil(hi[1])) + 1)
    if yhi <= ylo or xhi <= xlo:
        return [], []

    yext, xext = yhi - ylo, xhi - xlo
    # partition axis = smaller extent (fewer 128-blocks)
    transposed = xext < yext
    if transposed:
        p_lo0, p_ext_tot, f_lo0, f_hi0 = xlo, xext, ylo, yhi
        PAX, FAX = 1, 0
    else:
        p_lo0, p_ext_tot, f_lo0, f_hi0 = ylo, yext, xlo, xhi
        PAX, FAX = 0, 1

    invth = 1.0 / float(thick[b])
    tiles = []
    slots = []
    n_pb = (p_ext_tot + 127) // 128
    for pb in range(n_pb):
        p_lo = p_lo0 + pb * 128
        p_ext = min(128, p_lo0 + p_ext_tot - p_lo)
        # tile free extent = stroke free extent (shared across segments), even
        f_lo = f_lo0 & ~1
        f_ext = min(SIZE - f_lo, (f_hi0 - f_lo + 1) & ~1)
        tile = Tile(
            b, transposed, p_lo, p_ext, f_lo, f_ext,
            np.float32(invth * invth), np.float32(thick[b]),
        )
        tile_idx = None  # assigned by caller
        tiles.append(tile)

        p_c = p_lo + (p_ext - 1) / 2.0  # center of the partition block

        for s in range(P):
            vy, vx = v[s]
            wy, wx = w[s]
            dy, dx = wy - vy, wx - vx
            d2 = dy * dy + dx * dx
            d2p = d2 + 1e-5
            m = np.sqrt(d2p)
            h = m / 2.0
            inv_sd2p = 1.0 / m
            inv_sd2 = 1.0 / np.sqrt(d2) if d2 > 1e-4 else None

            # window on free axis: clip segment to this part-block's range
            vp, vf = (vy, vx) if not transposed else (vx, vy)
            wp, wf = (wy, wx) if not transposed else (wx, wy)
            blo, bhi = p_lo - margin, p_lo + p_ext - 1 + margin
            # param range of segment within [blo, bhi] on the partition axis
            if abs(wp - vp) < 1e-12:
                t0, t1 = 0.0, 1.0
                if vp < blo or vp > bhi:
                    continue
            else:
                ta = (blo - vp) / (wp - vp)
                tb = (bhi - vp) / (wp - vp)
                t0, t1 = max(0.0, min(ta, tb)), min(1.0, max(ta, tb))
                if t1 < t0:
                    continue
            fa = vf + t0 * (wf - vf)
            fb = vf + t1 * (wf - vf)
            w_lo = max(f_lo, int(np.floor(min(fa, fb) - margin)) & ~1)
            w_hi = min(f_lo + f_ext, (int(np.ceil(max(fa, fb) + margin)) + 2) & ~1)
            if w_hi <= w_lo:
                continue

            # affine coefficients: z and w planes over (p_hat, f)
            # dp/df = diff along partition/free axes
            dp, df = (dy, dx) if not transposed else (dx, dy)
            f = np.arange(w_lo, w_hi, dtype=np.float64)
            # s_dot(p, f) = (p - vp)*dp + (f - vf)*df ; p = p_c + p_hat
            # z = s_dot / m - h  (shifted so clamp window is [-h, h])
            if inv_sd2 is not None:
                row0_z = ((p_c - vp) * dp + (f - vf) * df) * inv_sd2p - h
                row1_z = dp * inv_sd2p
                # w_perp = ((p - vp)*df - (f - vf)*dp) / sqrt(d2)   (exact geometry)
                row0_w = ((p_c - vp) * df - (f - vf) * dp) * inv_sd2
                row1_w = df * inv_sd2
                h_clamp = h
            else:
                # degenerate (tiny) segment: point distance to v
                row0_z = (p_c - vp) + 0.0 * f
                row1_z = 1.0
                row0_w = f - vf
                row1_w = 0.0
                h_clamp = 0.0

            # split long windows into chunks <= MAX_W
            wdt = w_hi - w_lo
            n_ch = (wdt + MAX_W - 1) // MAX_W
            ch = ((wdt + n_ch - 1) // n_ch + 1) & ~1
            off = 0
            while off < wdt:
                cw = min(ch, wdt - off)
                slots.append(
                    Slot(
                        len(tiles) - 1,  # local tile index within this stroke
                        w_lo + off - f_lo,
                        cw,
                        row0_z[off : off + cw].astype(np.float32),
                        np.float32(row1_z),
                        row0_w[off : off + cw].astype(np.float32),
                        np.float32(row1_w),
                        np.float32(h_clamp),
                    )
                )
                off += cw
    return tiles, slots


def _plan_all(vs, ws, thick):
    """Plan tiles/slots for every stroke and greedily balance across cores."""
    per_stroke = []
    for b in range(B):
        tiles, slots = _plan_stroke(b, vs, ws, thick)
        cost = sum(232 + 3.5 * s.f_w for s in slots) + 800 * len(tiles)
        per_stroke.append((cost, b, tiles, slots))
    per_stroke.sort(reverse=True)
    core_cost = [0.0] * N_CORES
    core_work = [[] for _ in range(N_CORES)]  # list of (tiles, slots)
    for cost, b, tiles, slots in per_stroke:
        c = min(range(N_CORES), key=lambda i: core_cost[i])
        core_cost[c] += cost
        core_work[c].append((tiles, slots))
    return core_work


# ---------------------------------------------------------------------------
# bass program construction (one per core)
# ---------------------------------------------------------------------------
def _split_multiwait(nc, mybir):
    """This container's walrus accepts at most ONE semaphore wait per
    instruction; Tile attaches several.  Split extras onto NoOps."""
    for fn in nc.m.functions:
        for bb in fn.blocks:
            insts = bb.instructions
            idx = 0
            while idx < len(insts):
                inst = insts[idx]
                si = inst.sync_info
                ow = list(si.on_wait) if (si and si.on_wait) else []
                if len(ow) > 1:
                    si.on_wait = ow[-1:]
                    for j, w in enumerate(ow[:-1]):
                        nop = mybir.InstNoOp(
                            name=f"{inst.name}-ws{j}",
                            engine=inst.engine,
                            ins=[],
                            outs=[],
                            sync_info=mybir.SyncInfo(on_wait=[w], on_update=[]),
                        )
                        nc.register_instruction(nop, overwrite=True)
                        insts.insert(idx, nop)
                        idx += 1
                idx += 1


def _build_core_program(work):
    """work: list of (tiles, slots) per stroke.  Returns (nc, in_map, meta)."""
    import concourse.bass as bass
    import concourse.mybir as mybir
    import concourse.tile as tile_mod

    # flatten tiles; remap slot tile indices
    all_tiles = []
    all_slots = []
    for tiles, slots in work:
        base = len(all_tiles)
        all_tiles.extend(tiles)
        for s in slots:
            all_slots.append((base + s.tile_idx, s))
    n_tiles = max(1, len(all_tiles))

    # ---- pack host arrays ----
    # RHS [2, total_cols]: per slot 2*f_w cols: [z-half | w-half]
    # row0 = const row, row1 = p_hat coefficient (replicated)
    offs = []
    cols = 0
    for _, s in all_slots:
        offs.append(cols)
        cols += 2 * s.f_w
    cols = max(2, cols)
    rhs = np.zeros((2, cols), dtype=np.float32)
    for (ti, s), off in zip(all_slots, offs):
        fw = s.f_w
        rhs[0, off : off + fw] = s.row0_z
        rhs[1, off : off + fw] = s.row1_z
        rhs[0, off + fw : off + 2 * fw] = s.row0_w
        rhs[1, off + fw : off + 2 * fw] = s.row1_w

    # STAT [2, 128 * n_tiles]: per tile stationary [ones; p_hat]
    stat = np.zeros((2, 128 * n_tiles), dtype=np.float32)
    for t_i, t in enumerate(all_tiles):
        p_c = t.p_lo + (t.p_ext - 1) / 2.0
        stat[0, t_i * 128 : t_i * 128 + t.p_ext] = 1.0
        stat[1, t_i * 128 : t_i * 128 + t.p_ext] = (
            np.arange(t.p_lo, t.p_lo + t.p_ext, dtype=np.float64) - p_c
        ).astype(np.float32)

    # SCAL [128, n_slots + 2*n_tiles]: bcast scalars: per-slot h; per-tile invth2
    nscal = max(1, len(all_slots) + n_tiles)
    scal = np.zeros((128, nscal), dtype=np.float32)
    for i, (_, s) in enumerate(all_slots):
        scal[:, i] = s.h
    for t_i, t in enumerate(all_tiles):
        scal[:, len(all_slots) + t_i] = t.invth2

    # ---- trace program ----
    nc = bass.Bass()
    rhs_ext = nc.dram_tensor("rhs", list(rhs.shape), mybir.dt.float32, kind="ExternalInput")
    stat_ext = nc.dram_tensor("stat", list(stat.shape), mybir.dt.float32, kind="ExternalInput")
    scal_ext = nc.dram_tensor("scal", list(scal.shape), mybir.dt.float32, kind="ExternalInput")
    out_ext = nc.dram_tensor(
        "out", [n_tiles, 128, SIZE], mybir.dt.float32, kind="ExternalOutput"
    )

    with tile_mod.TileContext(nc) as tc:
        with ExitStack() as ctx:
            const_pool = ctx.enter_context(tc.tile_pool(name="const", bufs=1))
            minacc_pool = ctx.enter_context(tc.tile_pool(name="minacc", bufs=1))
            sb = ctx.enter_context(tc.tile_pool(name="work", bufs=6))
            psum = ctx.enter_context(tc.tile_pool(name="psum", bufs=4, space="PSUM"))
            outp = ctx.enter_context(tc.tile_pool(name="outp", bufs=2))

            t_rhs = const_pool.tile(list(rhs.shape), mybir.dt.float32)
            nc.gpsimd.dma_start(t_rhs[:], rhs_ext[:])
            t_stat = const_pool.tile(list(stat.shape), mybir.dt.float32)
            nc.gpsimd.dma_start(t_stat[:], stat_ext[:])
            t_scal = const_pool.tile(list(scal.shape), mybir.dt.float32)
            nc.gpsimd.dma_start(t_scal[:], scal_ext[:])

            # persistent min-accumulator per tile
            t_min = []
            for t_i, t in enumerate(all_tiles):
                m = minacc_pool.tile([128, t.f_ext], mybir.dt.float32, tag=f"min{t_i}")
                nc.vector.memset(m[:], INIT_MIN)
                t_min.append(m)

            # main sparse rasterization loop
            for i_slot, ((ti, s), off) in enumerate(zip(all_slots, offs)):
                t = all_tiles[ti]
                fw = s.f_w
                pe = t.p_ext
                zp = psum.tile([128, 2 * fw], mybir.dt.float32, tag="zp")
                nc.tensor.matmul(
                    zp[:pe, :],
                    t_stat[:, ti * 128 : ti * 128 + pe],
                    t_rhs[:, off : off + 2 * fw],
                    start=True,
                    stop=True,
                )
                a = sb.tile([128, fw], mybir.dt.float32, tag="a")
                nc.scalar.activation(a[:pe, :], zp[:pe, :fw], mybir.ActivationFunctionType.Abs)
                w2 = sb.tile([128, fw], mybir.dt.float32, tag="w2")
                nc.scalar.activation(
                    w2[:pe, :], zp[:pe, fw : 2 * fw], mybir.ActivationFunctionType.Square
                )
                e = sb.tile([128, fw], mybir.dt.float32, tag="e")
                nc.vector.tensor_scalar(
                    e[:pe, :], a[:pe, :], t_scal[:pe, i_slot : i_slot + 1], 0.0,
                    mybir.AluOpType.subtract, mybir.AluOpType.max,
                )
                e2 = sb.tile([128, fw], mybir.dt.float32, tag="e2")
                nc.vector.scalar_tensor_tensor(
                    e2[:pe, :], e[:pe, :], 1.0, e[:pe, :],
                    mybir.AluOpType.mult, mybir.AluOpType.mult,
                )
                d = sb.tile([128, fw], mybir.dt.float32, tag="d")
                nc.vector.tensor_tensor(d[:pe, :], e2[:pe, :], w2[:pe, :], mybir.AluOpType.add)
                msl = t_min[ti][:pe, s.f_lo : s.f_lo + fw]
                nc.vector.tensor_tensor(msl, msl, d[:pe, :], mybir.AluOpType.min)

            # epilogue per tile: dark = relu(1 - sqrt(minacc)/thick)
            for t_i, t in enumerate(all_tiles):
                pe = t.p_ext
                fe = t.f_ext
                sq = outp.tile([128, SIZE], mybir.dt.float32, tag="sq")
                nc.scalar.activation(
                    sq[:pe, :fe], t_min[t_i][:pe, :],
                    mybir.ActivationFunctionType.Sqrt,
                    scale=t_scal[:pe, len(all_slots) + t_i : len(all_slots) + t_i + 1],
                )
                dk = outp.tile([128, SIZE], mybir.dt.float32, tag="dk")
                nc.scalar.activation(
                    dk[:pe, :fe], sq[:pe, :fe],
                    mybir.ActivationFunctionType.Relu, bias=1.0, scale=-1.0,
                )
                nc.gpsimd.dma_start(out_ext[t_i, :pe, :fe], dk[:pe, :fe])

    _split_multiwait(nc, mybir)
    in_map = {"rhs": rhs, "stat": stat, "scal": scal}
    meta = all_tiles
    return nc, in_map, meta


# ---------------------------------------------------------------------------
# MPMD runner: one program per core, pinned via jax.default_device
# ---------------------------------------------------------------------------
def _make_exec(nc, in_map, device):
    """Build a cached jitted executor for one core's program. Returns run()
    -> dict of output arrays."""
    import jax
    import concourse.mybir as mybir
    from concourse import bass2jax

    bass2jax.install_neuronx_cc_hook()
    partition_name = nc.partition_id_tensor.name if nc.partition_id_tensor else None
    in_names, out_names, out_avals, zero_shapes = [], [], [], []
    for alloc in nc.m.functions[0].allocations:
        if not isinstance(alloc, mybir.MemoryLocationSet):
            continue
        name = alloc.memorylocations[0].name
        if alloc.kind == "ExternalInput":
            if name != partition_name:
                in_names.append(name)
        elif alloc.kind == "ExternalOutput":
            out_names.append(name)
            shape = tuple(alloc.tensor_shape)
            dtype = mybir.dt.np(alloc.dtype)
            out_avals.append(jax.core.ShapedArray(shape, dtype))
            zero_shapes.append((shape, dtype))
    n_params = len(in_names)
    all_in_names = list(in_names) + out_names
    if partition_name is not None:
        all_in_names.append(partition_name)
    donate = tuple(range(n_params, n_params + len(out_names)))

    def _body(*args):
        operands = list(args)
        if partition_name is not None:
            operands.append(bass2jax.partition_id_tensor())
        outs = bass2jax._bass_exec_p.bind(
            *operands,
            out_avals=tuple(out_avals),
            in_names=tuple(all_in_names),
            out_names=tuple(out_names),
            lowering_input_output_aliases=(),
            sim_require_finite=True,
            sim_require_nnan=True,
            nc=nc,
        )
        return tuple(outs)

    fn = jax.jit(_body, donate_argnums=donate, keep_unused=True)
    args = [np.asarray(in_map[n]) for n in in_names]

    def run(block=True):
        with jax.default_device(device):
            outs = fn(*args, *[np.zeros(s, d) for s, d in zero_shapes])
        if block:
            for o in outs:
                o.block_until_ready()
        return {name: outs[i] for i, name in enumerate(out_names)}

    return run


_CACHE = {}


def _prepare(trajectories, thicknesses):
    import jax

    key = (np.asarray(trajectories).tobytes(), np.asarray(thicknesses).tobytes())
    if key in _CACHE:
        return _CACHE[key]
    vs, ws, thick = _host_strokes(trajectories, thicknesses)
    core_work = _plan_all(vs, ws, thick)
    progs = [_build_core_program(core_work[c]) for c in range(N_CORES)]
    devices = jax.devices()[:N_CORES]
    runners = [None] * N_CORES
    errors = []

    def make(c):
        try:
            nc, in_map, _ = progs[c]
            runners[c] = _make_exec(nc, in_map, devices[c])
            runners[c]()  # warm up: compile + first exec
        except Exception as e:  # pragma: no cover
            errors.append((c, e))

    threads = [threading.Thread(target=make, args=(c,)) for c in range(N_CORES)]
    for t in threads:
        t.start()
    for t in threads:
        t.join()
    if errors:
        raise errors[0][1]
    _CACHE[key] = (progs, runners)
    return _CACHE[key]


def kernel(trajectories, thicknesses):
    trajectories = np.asarray(trajectories)
    thicknesses = np.asarray(thicknesses)
    progs, runners = _prepare(trajectories, thicknesses)

    results = [None] * N_CORES
    errors = []

    def runner(c):
        try:
            results[c] = runners[c]()
        except Exception as e:  # pragma: no cover
            errors.append((c, e))

    threads = [threading.Thread(target=runner, args=(c,)) for c in range(N_CORES)]
    for t in threads:
        t.start()
    for t in threads:
        t.join()
    if errors:
        raise errors[0][1]

    # assemble full output on host
    canvas = np.zeros((B, SIZE, SIZE), dtype=np.float32)
    for c in range(N_CORES):
        _, _, tiles = progs[c]
        out = np.asarray(results[c]["out"])
        for t_i, t in enumerate(tiles):
            block = out[t_i, : t.p_ext, : t.f_ext]
            if t.transposed:
                canvas[t.stroke, t.f_lo : t.f_lo + t.f_ext, t.p_lo : t.p_lo + t.p_ext] = block.T
            else:
                canvas[t.stroke, t.p_lo : t.p_lo + t.p_ext, t.f_lo : t.f_lo + t.f_ext] = block
    return canvas


def time_cores(inputs, repeats=30):
    """Amortized per-core execution time (seconds per exec)."""
    import time

    progs, runners = _prepare(**inputs)
    times = []
    for c in range(N_CORES):
        runners[c]()  # ensure warm
        t0 = time.time()
        for _ in range(repeats - 1):
            runners[c](block=False)
        runners[c](block=True)
        times.append((time.time() - t0) / repeats)
    return times
